# revision 6
# baseline (speedup 1.0000x reference)
"""Trainium2 Bass kernel for nn_ActorNetwork (moe_routing).

Design (host-routed expert parallelism, zero collectives):
  reference semantics: with perm = stable argsort(idx),
    h_f[i] = relu(relu(state[perm[i]] @ W1[g(i)] + b1[g(i)]) @ W2 + b2)
    out[i] = tanh(h_f[i] @ W3[idx[i]] + b3[idx[i]])
  where g(i) (the W1 expert of sorted-position i) depends only on which
  sorted-count block position i falls into.  Core c takes exactly the sorted
  block of game c -> its layer-1 is ONE dense matmul with only W1[c].  Within
  the core, rows are sub-grouped by head game idx[i] so layer-3 is 8 dense
  per-group matmuls.  All routing (gather of state rows in, scatter of output
  rows back) happens on the host during shard/unshard.  Groups are stored
  sorted by size (descending) so the SPMD-uniform slot capacities
  M_j = max_core(j-th largest group) give ~1% padding.

  On-device layout is feature-major: activations live as [feature, row] so
  every matmul is lhsT=weight-tile [K=128, M=128], rhs=activation [K=128,
  N=rows], PSUM out [M features, rows].  Compute dtype bf16, f32 PSUM.
  All inputs are packed on the host into a handful of [128, X] images so each
  one moves with a single large DMA (per-DMA overhead ~0.6us dominates small
  transfers).  State is packed chunk-major so each 512-row chunk is one DMA.
"""

import numpy as np
import ml_dtypes

_BF16 = ml_dtypes.bfloat16
_NCORES = 8
_WARM_MMS = 52  # PE warmup matmuls (~2.8us cold) to ramp HAM before real work
_graph_cache: dict = {}


def _make_plan(idx: np.ndarray, G: int):
    """Host routing plan: which (sorted-position) rows go to which core/slot."""
    idx = np.asarray(idx)
    perm = np.argsort(idx, kind="stable")
    counts = np.bincount(idx, minlength=G)
    cum = np.zeros(G + 1, dtype=np.int64)
    cum[1:] = np.cumsum(counts)

    core_groups = []  # per core: list of (head_game, sorted_positions) desc by size
    for c in range(G):
        pos = np.arange(cum[c], cum[c + 1])
        heads = idx[pos]
        groups = [(b, pos[heads == b]) for b in range(G)]
        groups.sort(key=lambda t: (-len(t[1]), t[0]))
        core_groups.append(groups)

    sizes = np.array([[len(p) for _, p in groups] for groups in core_groups])
    M = sizes.max(axis=0)          # slot capacity per position (SPMD-uniform)
    keep = M > 0
    M = M[keep]
    core_groups = [[g for g, k in zip(groups, keep) if k] for groups in core_groups]
    NG = len(M)
    starts = np.zeros(NG + 1, dtype=np.int64)
    starts[1:] = np.cumsum(M)
    N = int(starts[-1])
    return perm, core_groups, M, starts, N


def _chunks_of(N):
    out = []
    c0 = 0
    while c0 < N:
        cw = min(512, N - c0)
        out.append((c0, cw))
        c0 += cw
    return out


def _build_graph(D, H1, H2, A, NG, starts, N):
    """Build + finalize the SPMD Bass graph (identical for all cores)."""
    from concourse import bacc
    import concourse.mybir as mybir
    from concourse.tile import TileContext

    bf = mybir.dt.bfloat16
    f32 = mybir.dt.float32
    KD, K1, K2 = D // 128, H1 // 128, H2 // 128
    M1, M2 = H1 // 128, H2 // 128
    assert D % 128 == 0 and H1 % 128 == 0 and H2 % 128 == 0 and A == 128

    chunks = _chunks_of(N)
    # chunk-major packed state: chunk (c0, cw) occupies cols [c0*KD, (c0+cw)*KD)
    # with the (k, r) element at col c0*KD + k*cw + r.
    ST_W = N * KD

    nc = bacc.Bacc("TRN2")
    st_ext = nc.declare_dram_parameter("st", [128, ST_W], bf, isOutput=False)
    w1_ext = nc.declare_dram_parameter("w1", [128, KD * H1], bf, isOutput=False)
    w2_ext = nc.declare_dram_parameter("w2", [128, K1 * H2], bf, isOutput=False)
    w3_ext = nc.declare_dram_parameter("w3", [128, NG * K2 * A], bf, isOutput=False)
    b_ext = nc.declare_dram_parameter("bs", [128, M1 + M2 + NG], f32, isOutput=False)
    out_ext = nc.declare_dram_parameter("out", [A, N], f32, isOutput=True)

    add = mybir.AluOpType.add
    amax = mybir.AluOpType.max
    Tanh = mybir.ActivationFunctionType.Tanh

    with TileContext(nc) as tc:
        with (
            tc.tile_pool(name="weights", bufs=1) as wp,
            tc.tile_pool(name="acts", bufs=1) as ap,
            tc.tile_pool(name="psum", bufs=4, space="PSUM") as pp,
            tc.tile_pool(name="psum3", bufs=2, space="PSUM") as pp3,
            tc.tile_pool(name="psumw", bufs=1, space="PSUM") as ppw,
        ):
            stt = ap.tile([128, ST_W], bf, name="stt", tag="stt")
            w1t = wp.tile([128, KD * H1], bf, name="w1t", tag="w1t")
            w2t = wp.tile([128, K1 * H2], bf, name="w2t", tag="w2t")
            w3t = wp.tile([128, NG * K2 * A], bf, name="w3t", tag="w3t")
            bt = wp.tile([128, M1 + M2 + NG], f32, name="bt", tag="bt")
            h1 = [ap.tile([128, N], bf, name=f"h1_{m}", tag=f"h1_{m}") for m in range(M1)]
            hf = [ap.tile([128, N], bf, name=f"hf_{m}", tag=f"hf_{m}") for m in range(M2)]
            osb = ap.tile([A, N], f32, name="osb", tag="osb")
            wrm = wp.tile([128, 64], bf, name="wrm", tag="wrm")

            # PE warmup: ramp the HAM clock gate to 8/8 while DMAs land.
            nc.gpsimd.memset(wrm[:], 0)
            pw = ppw.tile([64, 64], f32, name="psw", tag="psw")
            for _ in range(_WARM_MMS):
                nc.tensor.matmul(pw[:], wrm[:, :64], wrm[:], start=True, stop=True)

            # DMAs: two HWDGE queues.  sync: biases + state (chunk-major, one
            # DMA per chunk).  scalar: w1 first (critical for first matmuls),
            # then w2, then w3 (ordered by first use).
            nc.sync.dma_start(bt[:], b_ext[:])
            for c0, cw in chunks:
                nc.sync.dma_start(
                    stt[:, c0 * KD : (c0 + cw) * KD], st_ext[:, c0 * KD : (c0 + cw) * KD]
                )
            nc.scalar.dma_start(w1t[:], w1_ext[:])
            nc.scalar.dma_start(w2t[:], w2_ext[:])
            nc.scalar.dma_start(w3t[:], w3_ext[:])

            done_j = 0
            for ci, (c0, cw) in enumerate(chunks):
                sl = slice(c0, c0 + cw)
                # L1: h1[m][:, sl] = relu(W1[:, m-tile].T @ st[:, sl] + b1)
                for m in range(M1):
                    ps = pp.tile([128, cw], f32, name="ps", tag="ps")
                    for k in range(KD):
                        nc.tensor.matmul(
                            ps[:],
                            w1t[:, k * H1 + m * 128 : k * H1 + (m + 1) * 128],
                            stt[:, c0 * KD + k * cw : c0 * KD + k * cw + cw],
                            start=(k == 0),
                            stop=(k == KD - 1),
                        )
                    nc.vector.tensor_scalar(
                        h1[m][:, sl], ps[:], bt[:, m : m + 1], 0.0, add, amax
                    )
                # L2
                for m in range(M2):
                    ps = pp.tile([128, cw], f32, name="ps", tag="ps")
                    for k in range(K1):
                        nc.tensor.matmul(
                            ps[:],
                            w2t[:, k * H2 + m * 128 : k * H2 + (m + 1) * 128],
                            h1[k][:, sl],
                            start=(k == 0),
                            stop=(k == K1 - 1),
                        )
                    nc.vector.tensor_scalar(
                        hf[m][:, sl], ps[:], bt[:, M1 + m : M1 + m + 1], 0.0, add, amax
                    )
                # L3 for fully-covered groups; batch the out-DMA per chunk.
                lim = c0 + cw
                first_j = done_j
                while done_j < NG and starts[done_j + 1] <= lim:
                    j = done_j
                    sj, ej = int(starts[j]), int(starts[j + 1])
                    mj = ej - sj
                    ps = pp3.tile([A, mj], f32, name="ps3", tag="ps3")
                    for k in range(K2):
                        nc.tensor.matmul(
                            ps[:],
                            w3t[:, j * K2 * A + k * A : j * K2 * A + (k + 1) * A],
                            hf[k][:, sj:ej],
                            start=(k == 0),
                            stop=(k == K2 - 1),
                        )
                    nc.scalar.activation(
                        osb[:, sj:ej], ps[:], Tanh, bias=bt[:, M1 + M2 + j : M1 + M2 + j + 1]
                    )
                    done_j += 1
                if done_j > first_j:
                    glo, ghi = int(starts[first_j]), int(starts[done_j])
                    nc.sync.dma_start(out_ext[:, glo:ghi], osb[:, glo:ghi])
            assert done_j == NG

    nc.finalize()
    return nc


def _kmajor(w, K):
    """[K*128, F] -> [128, K*F] with col = k*F + f."""
    F = w.shape[1]
    return np.ascontiguousarray(w.reshape(K, 128, F).transpose(1, 0, 2).reshape(128, K * F))


def _prepare(state, idx, W1, b1, W2, b2, W3, b3):
    state = np.ascontiguousarray(np.asarray(state, dtype=np.float32))
    idx = np.asarray(idx)
    W1 = np.asarray(W1, dtype=np.float32)
    b1 = np.asarray(b1, dtype=np.float32)
    W2 = np.asarray(W2, dtype=np.float32)
    b2 = np.asarray(b2, dtype=np.float32)
    W3 = np.asarray(W3, dtype=np.float32)
    b3 = np.asarray(b3, dtype=np.float32)

    B, D = state.shape
    G, _, H1 = W1.shape
    H2 = W2.shape[1]
    A = W3.shape[2]
    KD, K1, K2 = D // 128, H1 // 128, H2 // 128
    M1, M2 = H1 // 128, H2 // 128

    perm, core_groups, M, starts, N = _make_plan(idx, G)
    NG = len(M)
    chunks = _chunks_of(N)

    key = (D, H1, H2, A, NG, tuple(int(x) for x in starts), N)
    if key not in _graph_cache:
        _graph_cache[key] = _build_graph(D, H1, H2, A, NG, starts, N)
    nc = _graph_cache[key]

    w2_h = _kmajor(W2.astype(_BF16), K1)
    b2_col = b2.reshape(M2, 128).T.astype(np.float32)

    in_maps = []
    scatters = []  # per core: list of (sorted_positions, col_start)
    for c in range(G):
        sT = np.zeros((D, N), dtype=_BF16)
        w3_h = np.zeros((128, NG * K2 * A), dtype=_BF16)
        bs = np.zeros((128, M1 + M2 + NG), dtype=np.float32)
        bs[:, :M1] = b1[c].reshape(M1, 128).T
        bs[:, M1 : M1 + M2] = b2_col
        sc = []
        for j, (head, pos) in enumerate(core_groups[c]):
            s0 = int(starts[j])
            if len(pos):
                sT[:, s0 : s0 + len(pos)] = state[perm[pos]].T.astype(_BF16)
                sc.append((pos, s0))
            w3_h[:, j * K2 * A : (j + 1) * K2 * A] = (
                W3[head].astype(_BF16).reshape(K2, 128, A).transpose(1, 0, 2).reshape(128, K2 * A)
            )
            bs[:, M1 + M2 + j] = b3[head]
        # chunk-major state packing
        st_h = np.empty((128, N * KD), dtype=_BF16)
        for c0, cw in chunks:
            st_h[:, c0 * KD : (c0 + cw) * KD] = (
                sT[:, c0 : c0 + cw].reshape(KD, 128, cw).transpose(1, 0, 2).reshape(128, KD * cw)
            )
        in_maps.append(
            {
                "st": st_h,
                "w1": _kmajor(W1[c].astype(_BF16), KD),
                "w2": w2_h,
                "w3": w3_h,
                "bs": bs,
            }
        )
        scatters.append(sc)
    return nc, in_maps, scatters, (B, A)


def _run(state, idx, W1, b1, W2, b2, W3, b3, trace=False, trace_kwargs=None):
    from concourse.bass_utils import run_bass_kernel_spmd

    nc, in_maps, scatters, (B, A) = _prepare(state, idx, W1, b1, W2, b2, W3, b3)
    res = run_bass_kernel_spmd(
        nc,
        in_maps,
        core_ids=list(range(_NCORES)),
        trace=trace,
        **(trace_kwargs or {}),
    )
    out = np.zeros((B, A), dtype=np.float32)
    for c in range(len(scatters)):
        o = np.asarray(res.results[c]["out"], dtype=np.float32)  # [A, N]
        for pos, s0 in scatters[c]:
            out[pos] = o[:, s0 : s0 + len(pos)].T
    return out, res


def kernel(**inputs) -> np.ndarray:
    out, _ = _run(**inputs)
    return out


# revision 9
# speedup vs baseline: 1.1587x; 1.1587x over previous
"""Trainium2 Bass kernel for nn_ActorNetwork (moe_routing).

Design (host-routed expert parallelism, zero collectives):
  reference semantics: with perm = stable argsort(idx),
    h_f[i] = relu(relu(state[perm[i]] @ W1[g(i)] + b1[g(i)]) @ W2 + b2)
    out[i] = tanh(h_f[i] @ W3[idx[i]] + b3[idx[i]])
  where g(i) (the W1 expert of sorted-position i) depends only on which
  sorted-count block position i falls into.  Core c takes exactly the sorted
  block of game c -> its layer-1 is ONE dense matmul with only W1[c].  Within
  the core, rows are sub-grouped by head game idx[i] so layer-3 is 8 dense
  per-group matmuls.  All routing (gather of state rows in, scatter of output
  rows back) happens on the host during shard/unshard.  Groups are stored
  sorted by size (descending) so the SPMD-uniform slot capacities
  M_j = max_core(j-th largest group) give ~1% padding.

  On-device layout is feature-major: activations live as [feature, row] so
  every matmul is lhsT=weight-tile [K=128, M=128], rhs=activation [K=128,
  N=rows], PSUM out [M features, rows].  Compute dtype bf16, f32 PSUM.
  All inputs are packed on the host into a handful of [128, X] images so each
  one moves with a single large DMA (per-DMA overhead ~0.6us dominates small
  transfers).  State is packed chunk-major so each 512-row chunk is one DMA.
"""

import numpy as np
import ml_dtypes

_BF16 = ml_dtypes.bfloat16
_NCORES = 8
_WARM_MMS = 18  # PE warmup matmuls (~1us cold) to ramp HAM before real work
_graph_cache: dict = {}


def _make_plan(idx: np.ndarray, G: int):
    """Host routing plan: which (sorted-position) rows go to which core/slot."""
    idx = np.asarray(idx)
    perm = np.argsort(idx, kind="stable")
    counts = np.bincount(idx, minlength=G)
    cum = np.zeros(G + 1, dtype=np.int64)
    cum[1:] = np.cumsum(counts)

    core_groups = []  # per core: list of (head_game, sorted_positions) desc by size
    for c in range(G):
        pos = np.arange(cum[c], cum[c + 1])
        heads = idx[pos]
        groups = [(b, pos[heads == b]) for b in range(G)]
        groups.sort(key=lambda t: (-len(t[1]), t[0]))
        core_groups.append(groups)

    sizes = np.array([[len(p) for _, p in groups] for groups in core_groups])
    M = sizes.max(axis=0)          # slot capacity per position (SPMD-uniform)
    keep = M > 0
    M = M[keep]
    core_groups = [[g for g, k in zip(groups, keep) if k] for groups in core_groups]
    NG = len(M)
    starts = np.zeros(NG + 1, dtype=np.int64)
    starts[1:] = np.cumsum(M)
    N = int(starts[-1])
    return perm, core_groups, M, starts, N


def _chunks_of(N):
    import math
    nch = max(1, math.ceil(N / 512))
    base = N // nch
    rem = N - base * nch
    out = []
    c0 = 0
    for i in range(nch):
        cw = base + (1 if i < rem else 0)
        out.append((c0, cw))
        c0 += cw
    return out


def _build_graph(D, H1, H2, A, NG, starts, N):
    """Build + finalize the SPMD Bass graph (identical for all cores)."""
    from concourse import bacc
    import concourse.mybir as mybir
    from concourse.tile import TileContext

    bf = mybir.dt.bfloat16
    f32 = mybir.dt.float32
    KD, K1, K2 = D // 128, H1 // 128, H2 // 128
    M1, M2 = H1 // 128, H2 // 128
    assert D % 128 == 0 and H1 % 128 == 0 and H2 % 128 == 0 and A == 128

    chunks = _chunks_of(N)
    # chunk-major packed state: chunk (c0, cw) occupies cols [c0*KD, (c0+cw)*KD)
    # with the (k, r) element at col c0*KD + k*cw + r.
    ST_W = N * KD

    nc = bacc.Bacc("TRN2")
    st_ext = nc.declare_dram_parameter("st", [128, ST_W], bf, isOutput=False)
    w1_ext = nc.declare_dram_parameter("w1", [128, KD * H1], bf, isOutput=False)
    w2_ext = nc.declare_dram_parameter("w2", [128, K1 * H2], bf, isOutput=False)
    w3_ext = nc.declare_dram_parameter("w3", [128, NG * K2 * A], bf, isOutput=False)
    b_ext = nc.declare_dram_parameter("bs", [128, M1 + M2 + NG], f32, isOutput=False)
    out_ext = nc.declare_dram_parameter("out", [A, N], f32, isOutput=True)

    add = mybir.AluOpType.add
    amax = mybir.AluOpType.max
    Tanh = mybir.ActivationFunctionType.Tanh

    with TileContext(nc) as tc:
        with (
            tc.tile_pool(name="weights", bufs=1) as wp,
            tc.tile_pool(name="acts", bufs=1) as ap,
            tc.tile_pool(name="psum", bufs=4, space="PSUM") as pp,
            tc.tile_pool(name="psum3", bufs=2, space="PSUM") as pp3,
            tc.tile_pool(name="psumw", bufs=1, space="PSUM") as ppw,
        ):
            stt = ap.tile([128, ST_W], bf, name="stt", tag="stt")
            w1t = wp.tile([128, KD * H1], bf, name="w1t", tag="w1t")
            w2t = wp.tile([128, K1 * H2], bf, name="w2t", tag="w2t")
            w3t = wp.tile([128, NG * K2 * A], bf, name="w3t", tag="w3t")
            bt = wp.tile([128, M1 + M2 + NG], f32, name="bt", tag="bt")
            h1 = [ap.tile([128, N], bf, name=f"h1_{m}", tag=f"h1_{m}") for m in range(M1)]
            hf = [ap.tile([128, N], bf, name=f"hf_{m}", tag=f"hf_{m}") for m in range(M2)]
            osb = ap.tile([A, N], f32, name="osb", tag="osb")
            wrm = wp.tile([128, 64], bf, name="wrm", tag="wrm")

            # PE warmup: ramp the HAM clock gate to 8/8 while DMAs land.
            nc.gpsimd.memset(wrm[:], 0)
            pw = ppw.tile([64, 64], f32, name="psw", tag="psw")
            for _ in range(_WARM_MMS):
                nc.tensor.matmul(pw[:], wrm[:, :64], wrm[:], start=True, stop=True)

            # DMAs on two HWDGE queues, split so the first-wave pieces (k0 of
            # chunk 0 state, k0 of w1) land after only ~350KB.
            c0w = chunks[0][1]
            nc.sync.dma_start(bt[:], b_ext[:])
            nc.sync.dma_start(stt[:, 0:c0w], st_ext[:, 0:c0w])            # c0 k0
            nc.scalar.dma_start(w1t[:, 0:H1], w1_ext[:, 0:H1])            # w1 k0
            nc.sync.dma_start(stt[:, c0w : KD * c0w], st_ext[:, c0w : KD * c0w])
            nc.scalar.dma_start(w1t[:, H1:], w1_ext[:, H1:])
            half2 = (K1 // 2) * H2
            nc.scalar.dma_start(w2t[:, 0:half2], w2_ext[:, 0:half2])
            for c0, cw in chunks[1:]:
                nc.sync.dma_start(
                    stt[:, c0 * KD : (c0 + cw) * KD], st_ext[:, c0 * KD : (c0 + cw) * KD]
                )
            nc.scalar.dma_start(w2t[:, half2:], w2_ext[:, half2:])
            nc.scalar.dma_start(w3t[:], w3_ext[:])

            def l1_mm(ps, m, k, c0, cw):
                nc.tensor.matmul(
                    ps[:],
                    w1t[:, k * H1 + m * 128 : k * H1 + (m + 1) * 128],
                    stt[:, c0 * KD + k * cw : c0 * KD + k * cw + cw],
                    start=(k == 0),
                    stop=(k == KD - 1),
                )

            def l2_mm(ps, m, k, sl):
                nc.tensor.matmul(
                    ps[:],
                    w2t[:, k * H2 + m * 128 : k * H2 + (m + 1) * 128],
                    h1[k][:, sl],
                    start=(k == 0),
                    stop=(k == K1 - 1),
                )

            def relu1(ps, m, sl):
                nc.vector.tensor_scalar(
                    h1[m][:, sl], ps[:], bt[:, m : m + 1], 0.0, add, amax
                )

            def relu2(ps, m, sl):
                nc.vector.tensor_scalar(
                    hf[m][:, sl], ps[:], bt[:, M1 + m : M1 + m + 1], 0.0, add, amax
                )

            done_j = 0
            for ci, (c0, cw) in enumerate(chunks):
                sl = slice(c0, c0 + cw)
                if ci == 0:
                    # k-outer waves of 4 m-tiles: consume st/w1/w2 k-slices as
                    # the split DMAs deliver them, so PE starts ~350KB in.
                    for wave in (range(0, 4), range(4, M1)):
                        pss = [pp.tile([128, cw], f32, name="ps", tag="ps") for _ in wave]
                        for k in range(KD):
                            for i, m in enumerate(wave):
                                l1_mm(pss[i], m, k, c0, cw)
                        for i, m in enumerate(wave):
                            relu1(pss[i], m, sl)
                    for wave in (range(0, 4), range(4, M2)):
                        pss = [pp.tile([128, cw], f32, name="ps", tag="ps") for _ in wave]
                        for k in range(K1):
                            for i, m in enumerate(wave):
                                l2_mm(pss[i], m, k, sl)
                        for i, m in enumerate(wave):
                            relu2(pss[i], m, sl)
                else:
                    for m in range(M1):
                        ps = pp.tile([128, cw], f32, name="ps", tag="ps")
                        for k in range(KD):
                            l1_mm(ps, m, k, c0, cw)
                        relu1(ps, m, sl)
                    for m in range(M2):
                        ps = pp.tile([128, cw], f32, name="ps", tag="ps")
                        for k in range(K1):
                            l2_mm(ps, m, k, sl)
                        relu2(ps, m, sl)
                # L3 for fully-covered groups; batch the out-DMA per chunk.
                lim = c0 + cw
                first_j = done_j
                while done_j < NG and starts[done_j + 1] <= lim:
                    j = done_j
                    sj, ej = int(starts[j]), int(starts[j + 1])
                    mj = ej - sj
                    ps = pp3.tile([A, mj], f32, name="ps3", tag="ps3")
                    for k in range(K2):
                        nc.tensor.matmul(
                            ps[:],
                            w3t[:, j * K2 * A + k * A : j * K2 * A + (k + 1) * A],
                            hf[k][:, sj:ej],
                            start=(k == 0),
                            stop=(k == K2 - 1),
                        )
                    nc.scalar.activation(
                        osb[:, sj:ej], ps[:], Tanh, bias=bt[:, M1 + M2 + j : M1 + M2 + j + 1]
                    )
                    done_j += 1
                if done_j > first_j:
                    glo, ghi = int(starts[first_j]), int(starts[done_j])
                    nc.sync.dma_start(out_ext[:, glo:ghi], osb[:, glo:ghi])
            assert done_j == NG

    nc.finalize()
    return nc


def _kmajor(w, K):
    """[K*128, F] -> [128, K*F] with col = k*F + f."""
    F = w.shape[1]
    return np.ascontiguousarray(w.reshape(K, 128, F).transpose(1, 0, 2).reshape(128, K * F))


def _prepare(state, idx, W1, b1, W2, b2, W3, b3):
    state = np.ascontiguousarray(np.asarray(state, dtype=np.float32))
    idx = np.asarray(idx)
    W1 = np.asarray(W1, dtype=np.float32)
    b1 = np.asarray(b1, dtype=np.float32)
    W2 = np.asarray(W2, dtype=np.float32)
    b2 = np.asarray(b2, dtype=np.float32)
    W3 = np.asarray(W3, dtype=np.float32)
    b3 = np.asarray(b3, dtype=np.float32)

    B, D = state.shape
    G, _, H1 = W1.shape
    H2 = W2.shape[1]
    A = W3.shape[2]
    KD, K1, K2 = D // 128, H1 // 128, H2 // 128
    M1, M2 = H1 // 128, H2 // 128

    perm, core_groups, M, starts, N = _make_plan(idx, G)
    NG = len(M)
    chunks = _chunks_of(N)

    key = (D, H1, H2, A, NG, tuple(int(x) for x in starts), N)
    if key not in _graph_cache:
        _graph_cache[key] = _build_graph(D, H1, H2, A, NG, starts, N)
    nc = _graph_cache[key]

    w2_h = _kmajor(W2.astype(_BF16), K1)
    b2_col = b2.reshape(M2, 128).T.astype(np.float32)

    in_maps = []
    scatters = []  # per core: list of (sorted_positions, col_start)
    for c in range(G):
        sT = np.zeros((D, N), dtype=_BF16)
        w3_h = np.zeros((128, NG * K2 * A), dtype=_BF16)
        bs = np.zeros((128, M1 + M2 + NG), dtype=np.float32)
        bs[:, :M1] = b1[c].reshape(M1, 128).T
        bs[:, M1 : M1 + M2] = b2_col
        sc = []
        for j, (head, pos) in enumerate(core_groups[c]):
            s0 = int(starts[j])
            if len(pos):
                sT[:, s0 : s0 + len(pos)] = state[perm[pos]].T.astype(_BF16)
                sc.append((pos, s0))
            w3_h[:, j * K2 * A : (j + 1) * K2 * A] = (
                W3[head].astype(_BF16).reshape(K2, 128, A).transpose(1, 0, 2).reshape(128, K2 * A)
            )
            bs[:, M1 + M2 + j] = b3[head]
        # chunk-major state packing
        st_h = np.empty((128, N * KD), dtype=_BF16)
        for c0, cw in chunks:
            st_h[:, c0 * KD : (c0 + cw) * KD] = (
                sT[:, c0 : c0 + cw].reshape(KD, 128, cw).transpose(1, 0, 2).reshape(128, KD * cw)
            )
        in_maps.append(
            {
                "st": st_h,
                "w1": _kmajor(W1[c].astype(_BF16), KD),
                "w2": w2_h,
                "w3": w3_h,
                "bs": bs,
            }
        )
        scatters.append(sc)
    return nc, in_maps, scatters, (B, A)


def _run(state, idx, W1, b1, W2, b2, W3, b3, trace=False, trace_kwargs=None):
    from concourse.bass_utils import run_bass_kernel_spmd

    nc, in_maps, scatters, (B, A) = _prepare(state, idx, W1, b1, W2, b2, W3, b3)
    res = run_bass_kernel_spmd(
        nc,
        in_maps,
        core_ids=list(range(_NCORES)),
        trace=trace,
        **(trace_kwargs or {}),
    )
    out = np.zeros((B, A), dtype=np.float32)
    for c in range(len(scatters)):
        o = np.asarray(res.results[c]["out"], dtype=np.float32)  # [A, N]
        for pos, s0 in scatters[c]:
            out[pos] = o[:, s0 : s0 + len(pos)].T
    return out, res


def kernel(**inputs) -> np.ndarray:
    out, _ = _run(**inputs)
    return out


# revision 11
# speedup vs baseline: 1.1966x; 1.0327x over previous
"""Trainium2 Bass kernel for nn_ActorNetwork (moe_routing).

Design (host-routed expert parallelism, zero collectives):
  reference semantics: with perm = stable argsort(idx),
    h_f[i] = relu(relu(state[perm[i]] @ W1[g(i)] + b1[g(i)]) @ W2 + b2)
    out[i] = tanh(h_f[i] @ W3[idx[i]] + b3[idx[i]])
  where g(i) (the W1 expert of sorted-position i) depends only on which
  sorted-count block position i falls into.  Core c takes exactly the sorted
  block of game c -> its layer-1 is ONE dense matmul with only W1[c].  Within
  the core, rows are sub-grouped by head game idx[i] so layer-3 is 8 dense
  per-group matmuls.  All routing (gather of state rows in, scatter of output
  rows back) happens on the host during shard/unshard.  Groups are stored
  sorted by size (descending) so the SPMD-uniform slot capacities
  M_j = max_core(j-th largest group) give ~1% padding.

  On-device layout is feature-major: activations live as [feature, row] so
  every matmul is lhsT=weight-tile [K=128, M=128], rhs=activation [K=128,
  N=rows], PSUM out [M features, rows].  Compute dtype bf16, f32 PSUM.
  All inputs are packed on the host into a handful of [128, X] images so each
  one moves with a single large DMA (per-DMA overhead ~0.6us dominates small
  transfers).  State is packed chunk-major so each 512-row chunk is one DMA.
"""

import numpy as np
import ml_dtypes

_BF16 = ml_dtypes.bfloat16
_NCORES = 8
_WARM_MMS = 18  # PE warmup matmuls (~1us cold) to ramp HAM before real work
_graph_cache: dict = {}


def _make_plan(idx: np.ndarray, G: int):
    """Host routing plan: which (sorted-position) rows go to which core/slot."""
    idx = np.asarray(idx)
    perm = np.argsort(idx, kind="stable")
    counts = np.bincount(idx, minlength=G)
    cum = np.zeros(G + 1, dtype=np.int64)
    cum[1:] = np.cumsum(counts)

    core_groups = []  # per core: list of (head_game, sorted_positions) desc by size
    for c in range(G):
        pos = np.arange(cum[c], cum[c + 1])
        heads = idx[pos]
        groups = [(b, pos[heads == b]) for b in range(G)]
        groups.sort(key=lambda t: (-len(t[1]), t[0]))
        core_groups.append(groups)

    sizes = np.array([[len(p) for _, p in groups] for groups in core_groups])
    M = sizes.max(axis=0)          # slot capacity per position (SPMD-uniform)
    keep = M > 0
    M = M[keep]
    core_groups = [[g for g, k in zip(groups, keep) if k] for groups in core_groups]
    NG = len(M)
    starts = np.zeros(NG + 1, dtype=np.int64)
    starts[1:] = np.cumsum(M)
    N = int(starts[-1])
    return perm, core_groups, M, starts, N


def _chunks_of(N):
    import math
    nch = max(1, math.ceil(N / 512))
    base = N // nch
    rem = N - base * nch
    out = []
    c0 = 0
    for i in range(nch):
        cw = base + (1 if i < rem else 0)
        out.append((c0, cw))
        c0 += cw
    return out


def _build_graph(D, H1, H2, A, NG, starts, N):
    """Build + finalize the SPMD Bass graph (identical for all cores)."""
    from concourse import bacc
    import concourse.mybir as mybir
    from concourse.tile import TileContext

    bf = mybir.dt.bfloat16
    f32 = mybir.dt.float32
    KD, K1, K2 = D // 128, H1 // 128, H2 // 128
    M1, M2 = H1 // 128, H2 // 128
    assert D % 128 == 0 and H1 % 128 == 0 and H2 % 128 == 0 and A == 128

    chunks = _chunks_of(N)
    # chunk-major packed state: chunk (c0, cw) occupies cols [c0*KD, (c0+cw)*KD)
    # with the (k, r) element at col c0*KD + k*cw + r.
    ST_W = N * KD

    nc = bacc.Bacc("TRN2")
    st_ext = nc.declare_dram_parameter("st", [128, ST_W], bf, isOutput=False)
    w1_ext = nc.declare_dram_parameter("w1", [128, KD * H1], bf, isOutput=False)
    w2_ext = nc.declare_dram_parameter("w2", [128, K1 * H2], bf, isOutput=False)
    w3_ext = nc.declare_dram_parameter("w3", [128, NG * K2 * A], bf, isOutput=False)
    b_ext = nc.declare_dram_parameter("bs", [128, M1 + M2 + NG], f32, isOutput=False)
    out_ext = nc.declare_dram_parameter("out", [A, N], f32, isOutput=True)

    add = mybir.AluOpType.add
    amax = mybir.AluOpType.max
    Tanh = mybir.ActivationFunctionType.Tanh

    with TileContext(nc) as tc:
        with (
            tc.tile_pool(name="weights", bufs=1) as wp,
            tc.tile_pool(name="acts", bufs=1) as ap,
            tc.tile_pool(name="psum", bufs=4, space="PSUM") as pp,
            tc.tile_pool(name="psum3", bufs=2, space="PSUM") as pp3,
            tc.tile_pool(name="psumw", bufs=1, space="PSUM") as ppw,
        ):
            stt = ap.tile([128, ST_W], bf, name="stt", tag="stt")
            w1t = wp.tile([128, KD * H1], bf, name="w1t", tag="w1t")
            w2t = wp.tile([128, K1 * H2], bf, name="w2t", tag="w2t")
            w3t = wp.tile([128, NG * K2 * A], bf, name="w3t", tag="w3t")
            bt = wp.tile([128, M1 + M2 + NG], f32, name="bt", tag="bt")
            h1 = [ap.tile([128, N], bf, name=f"h1_{m}", tag=f"h1_{m}") for m in range(M1)]
            hf = [ap.tile([128, N], bf, name=f"hf_{m}", tag=f"hf_{m}") for m in range(M2)]
            osb = ap.tile([A, N], f32, name="osb", tag="osb")
            wrm = wp.tile([128, 64], bf, name="wrm", tag="wrm")

            # PE warmup: ramp the HAM clock gate to 8/8 while DMAs land.
            nc.gpsimd.memset(wrm[:], 0)
            pw = ppw.tile([64, 64], f32, name="psw", tag="psw")
            for _ in range(_WARM_MMS):
                nc.tensor.matmul(pw[:], wrm[:, :64], wrm[:], start=True, stop=True)

            # DMAs on two HWDGE queues.  Per-k slices of chunk-0 state (sync)
            # and w1 (scalar) land pairwise so L1 k-waves start ~350KB in;
            # w2/w3 stream behind while all L1 chunks run.
            c0w = chunks[0][1]
            nc.sync.dma_start(bt[:], b_ext[:])
            for k in range(KD):
                nc.sync.dma_start(
                    stt[:, k * c0w : (k + 1) * c0w], st_ext[:, k * c0w : (k + 1) * c0w]
                )
                nc.scalar.dma_start(
                    w1t[:, k * H1 : (k + 1) * H1], w1_ext[:, k * H1 : (k + 1) * H1]
                )
            for c0, cw in chunks[1:]:
                nc.sync.dma_start(
                    stt[:, c0 * KD : (c0 + cw) * KD], st_ext[:, c0 * KD : (c0 + cw) * KD]
                )
            half2 = (K1 // 2) * H2
            nc.scalar.dma_start(w2t[:, 0:half2], w2_ext[:, 0:half2])
            nc.scalar.dma_start(w2t[:, half2:], w2_ext[:, half2:])
            nc.scalar.dma_start(w3t[:], w3_ext[:])

            def l1_mm(ps, m, k, c0, cw):
                nc.tensor.matmul(
                    ps[:],
                    w1t[:, k * H1 + m * 128 : k * H1 + (m + 1) * 128],
                    stt[:, c0 * KD + k * cw : c0 * KD + k * cw + cw],
                    start=(k == 0),
                    stop=(k == KD - 1),
                )

            def l2_mm(ps, m, k, sl):
                nc.tensor.matmul(
                    ps[:],
                    w2t[:, k * H2 + m * 128 : k * H2 + (m + 1) * 128],
                    h1[k][:, sl],
                    start=(k == 0),
                    stop=(k == K1 - 1),
                )

            def relu1(ps, m, sl):
                nc.vector.tensor_scalar(
                    h1[m][:, sl], ps[:], bt[:, m : m + 1], 0.0, add, amax
                )

            def relu2(ps, m, sl):
                nc.vector.tensor_scalar(
                    hf[m][:, sl], ps[:], bt[:, M1 + m : M1 + m + 1], 0.0, add, amax
                )

            # Phase 1: L1 for ALL chunks (needs only st+w1 ~2.1MB) so w2/w3
            # stream in behind the compute.  Chunk 0 runs k-outer in waves of
            # 4 m-tiles to start as soon as the first k-slices land.
            for ci, (c0, cw) in enumerate(chunks):
                sl = slice(c0, c0 + cw)
                if ci == 0:
                    for wave in (range(0, 4), range(4, M1)):
                        pss = [pp.tile([128, cw], f32, name="ps", tag="ps") for _ in wave]
                        for k in range(KD):
                            for i, m in enumerate(wave):
                                l1_mm(pss[i], m, k, c0, cw)
                        for i, m in enumerate(wave):
                            relu1(pss[i], m, sl)
                else:
                    for m in range(M1):
                        ps = pp.tile([128, cw], f32, name="ps", tag="ps")
                        for k in range(KD):
                            l1_mm(ps, m, k, c0, cw)
                        relu1(ps, m, sl)
            # Phase 2+3: L2 per chunk, then L3 for fully-covered groups with
            # one batched out-DMA per chunk.
            done_j = 0
            for ci, (c0, cw) in enumerate(chunks):
                sl = slice(c0, c0 + cw)
                for m in range(M2):
                    ps = pp.tile([128, cw], f32, name="ps", tag="ps")
                    for k in range(K1):
                        l2_mm(ps, m, k, sl)
                    relu2(ps, m, sl)
                lim = c0 + cw
                first_j = done_j
                while done_j < NG and starts[done_j + 1] <= lim:
                    j = done_j
                    sj, ej = int(starts[j]), int(starts[j + 1])
                    mj = ej - sj
                    ps = pp3.tile([A, mj], f32, name="ps3", tag="ps3")
                    for k in range(K2):
                        nc.tensor.matmul(
                            ps[:],
                            w3t[:, j * K2 * A + k * A : j * K2 * A + (k + 1) * A],
                            hf[k][:, sj:ej],
                            start=(k == 0),
                            stop=(k == K2 - 1),
                        )
                    nc.scalar.activation(
                        osb[:, sj:ej], ps[:], Tanh, bias=bt[:, M1 + M2 + j : M1 + M2 + j + 1]
                    )
                    done_j += 1
                if done_j > first_j:
                    glo, ghi = int(starts[first_j]), int(starts[done_j])
                    nc.sync.dma_start(out_ext[:, glo:ghi], osb[:, glo:ghi])
            assert done_j == NG

    nc.finalize()
    return nc


def _kmajor(w, K):
    """[K*128, F] -> [128, K*F] with col = k*F + f."""
    F = w.shape[1]
    return np.ascontiguousarray(w.reshape(K, 128, F).transpose(1, 0, 2).reshape(128, K * F))


def _prepare(state, idx, W1, b1, W2, b2, W3, b3):
    state = np.ascontiguousarray(np.asarray(state, dtype=np.float32))
    idx = np.asarray(idx)
    W1 = np.asarray(W1, dtype=np.float32)
    b1 = np.asarray(b1, dtype=np.float32)
    W2 = np.asarray(W2, dtype=np.float32)
    b2 = np.asarray(b2, dtype=np.float32)
    W3 = np.asarray(W3, dtype=np.float32)
    b3 = np.asarray(b3, dtype=np.float32)

    B, D = state.shape
    G, _, H1 = W1.shape
    H2 = W2.shape[1]
    A = W3.shape[2]
    KD, K1, K2 = D // 128, H1 // 128, H2 // 128
    M1, M2 = H1 // 128, H2 // 128

    perm, core_groups, M, starts, N = _make_plan(idx, G)
    NG = len(M)
    chunks = _chunks_of(N)

    key = (D, H1, H2, A, NG, tuple(int(x) for x in starts), N)
    if key not in _graph_cache:
        _graph_cache[key] = _build_graph(D, H1, H2, A, NG, starts, N)
    nc = _graph_cache[key]

    w2_h = _kmajor(W2.astype(_BF16), K1)
    b2_col = b2.reshape(M2, 128).T.astype(np.float32)

    in_maps = []
    scatters = []  # per core: list of (sorted_positions, col_start)
    for c in range(G):
        sT = np.zeros((D, N), dtype=_BF16)
        w3_h = np.zeros((128, NG * K2 * A), dtype=_BF16)
        bs = np.zeros((128, M1 + M2 + NG), dtype=np.float32)
        bs[:, :M1] = b1[c].reshape(M1, 128).T
        bs[:, M1 : M1 + M2] = b2_col
        sc = []
        for j, (head, pos) in enumerate(core_groups[c]):
            s0 = int(starts[j])
            if len(pos):
                sT[:, s0 : s0 + len(pos)] = state[perm[pos]].T.astype(_BF16)
                sc.append((pos, s0))
            w3_h[:, j * K2 * A : (j + 1) * K2 * A] = (
                W3[head].astype(_BF16).reshape(K2, 128, A).transpose(1, 0, 2).reshape(128, K2 * A)
            )
            bs[:, M1 + M2 + j] = b3[head]
        # chunk-major state packing
        st_h = np.empty((128, N * KD), dtype=_BF16)
        for c0, cw in chunks:
            st_h[:, c0 * KD : (c0 + cw) * KD] = (
                sT[:, c0 : c0 + cw].reshape(KD, 128, cw).transpose(1, 0, 2).reshape(128, KD * cw)
            )
        in_maps.append(
            {
                "st": st_h,
                "w1": _kmajor(W1[c].astype(_BF16), KD),
                "w2": w2_h,
                "w3": w3_h,
                "bs": bs,
            }
        )
        scatters.append(sc)
    return nc, in_maps, scatters, (B, A)


def _run(state, idx, W1, b1, W2, b2, W3, b3, trace=False, trace_kwargs=None):
    from concourse.bass_utils import run_bass_kernel_spmd

    nc, in_maps, scatters, (B, A) = _prepare(state, idx, W1, b1, W2, b2, W3, b3)
    res = run_bass_kernel_spmd(
        nc,
        in_maps,
        core_ids=list(range(_NCORES)),
        trace=trace,
        **(trace_kwargs or {}),
    )
    out = np.zeros((B, A), dtype=np.float32)
    for c in range(len(scatters)):
        o = np.asarray(res.results[c]["out"], dtype=np.float32)  # [A, N]
        for pos, s0 in scatters[c]:
            out[pos] = o[:, s0 : s0 + len(pos)].T
    return out, res


def kernel(**inputs) -> np.ndarray:
    out, _ = _run(**inputs)
    return out
